# revision 23
# baseline (speedup 1.0000x reference)
"""Trainium2 Bass kernel for batched dense attention.

Reference computation (per batch b):
    q = query @ Wq + bq ; k = key @ Wk + bk ; v = value @ Wv + bv
    out = softmax(BETA * q k^T) v

Shapes: query/key/value [4, 2048, 1024], weights [1024, 1024], out [4, 2048, 1024].

Sharding: 8 cores = (batch b, seq half h). Each core computes out rows
[b, h*1024:(h+1)*1024, :] from its query shard [1024, 1024] plus the full
key/value of its batch (no collectives).

Algebraic restructure vs the naive form (reaches the ideal 7.52 GMAC/core):
  - scores = (query Wq + bq) (key Wk + bk)^T. The bk term contributes a
    constant per query row, which cancels exactly in softmax, so
    scores ~ qp (Wk^T) key^T with qp = query Wq + bq. The K projection
    (which would be duplicated across the 2 cores of a batch) is replaced
    by the q-side fold tT = WkT.T @ qpT.
  - A @ value is computed against the RAW value (V projection deferred):
    out = (p.T value) Wv * (1/rowsum) + bv, with the softmax normalization
    applied at the very end (linear).

All matmuls run in bf16 (f32 PSUM accumulation): bf16 weights get the
hardware fast-weight-load path (fp32/f32r weights do not), so every
512-row matmul streams at the ~259 ns cadence floor. Measured precision
is a few 1e-3 relative, well inside the 2e-2 gate.

Dataflow (per core), all layouts chosen so matmul operands are natural:
  P1: queryT (PE transpose) ; qpT[kd,q] = Wq-chunks.T @ queryT + bq
  P2: WkT (PE transpose of Wk) ; tT[d,q] = WkT.T @ qpT
  P3: keyT[d,k] (PE transpose of key) ; value resident bf16 (natural),
      Wv resident bf16 (natural)
  A:  sT[k,q-blk] = keyT.T @ tT ; exp(BETA*sT) -> pT bf16 ;
      rowsums via tiny PE matmuls with ones
  B:  o2T[d,q-blk] = value-chunk.T @ pT (o2T produced directly, no
      transposes)
  C:  out = (o2T.T @ Wv) * (1/rowsum) + bv, DMA out
"""
import numpy as np

import concourse.bass as bass
import concourse.bacc as bacc
import concourse.tile as tile
from concourse import masks, mybir
from concourse.bass_utils import run_bass_kernel_spmd

B, S, D = 4, 2048, 1024
KD = 1024  # key_dim == value_dim == D
VD = 1024
BETA = 1.0 / float(np.sqrt(D))
N_CORES = 8
QS = S // 2  # per-core query rows (1024)

F32 = mybir.dt.float32
F32R = mybir.dt.float32r
BF16 = mybir.dt.bfloat16

C_D = D // 128     # 8 chunks over D (value cols / score contraction)
C_KD = KD // 128   # 8 chunks over KD
KT = S // 128      # 16 key tiles
QBLK = 512         # q-block size
NQB = QS // QBLK   # 2 q blocks
NQS = QBLK // 128  # 4 q slices per block


def build_kernel():
    nc = bacc.Bacc("TRN2", target_bir_lowering=False, debug=False,
                   num_devices=N_CORES)

    q_sh = nc.dram_tensor("q_sh", [QS, D], F32, kind="ExternalInput").ap()
    key_b = nc.dram_tensor("key_b", [S, D], F32, kind="ExternalInput").ap()
    val_b = nc.dram_tensor("val_b", [S, D], F32, kind="ExternalInput").ap()
    Wq = nc.dram_tensor("Wq", [D, KD], F32, kind="ExternalInput").ap()
    Wk = nc.dram_tensor("Wk", [D, KD], F32, kind="ExternalInput").ap()
    Wv = nc.dram_tensor("Wv", [D, VD], F32, kind="ExternalInput").ap()
    bq = nc.dram_tensor("bq", [KD], F32, kind="ExternalInput").ap()
    bk = nc.dram_tensor("bk", [KD], F32, kind="ExternalInput").ap()
    bv = nc.dram_tensor("bv", [VD], F32, kind="ExternalInput").ap()
    out = nc.dram_tensor("out", [QS, VD], F32, kind="ExternalOutput").ap()

    with tile.TileContext(nc) as tc:
        _body(tc, q_sh, key_b, val_b, Wq, Wk, Wv, bq, bv, out)
    nc.compile()
    return nc


def _body(tc, q_sh, key_b, val_b, Wq, Wk, Wv, bq, bv, out):
    nc = tc.nc
    Exp = mybir.ActivationFunctionType.Exp
    mult = mybir.AluOpType.mult
    add = mybir.AluOpType.add

    # ---- consolidated persistent constants ------------------------------
    # constf cols: [0:8]=bqT, [8:8+VD]=bvb, [1036:1038] ones cols,
    # row0 [1040:1168] onesrow staging, [1168:] rrec columns
    const_pool = tc.alloc_tile_pool(name="const", bufs=1)
    constf = const_pool.tile([128, 1184], F32, name="constf")
    bqT = constf[:, 0:8]
    bvb = constf[:, 8:8 + VD]
    ones_f = constf[:, 1036:1038]
    onesrow_f = constf[0:1, 1040:1040 + 128]
    bv_f = constf[0:1, 8:8 + VD]
    rrec_all = constf[:, 1168:1168 + 2 * (QS // 128)]
    # constr: row0 [0:VD]=bv_r, [1028:1028+128]=onesrow_r
    constr = const_pool.tile([128, 1184], F32R, name="constr")
    bv_r = constr[0:1, 0:VD]
    onesrow_r = constr[0:1, 1028:1028 + 128]
    onesb = const_pool.tile([128, 2], BF16, name="onesb")

    # bq stages through the bvb row-0 area first (one fast row DMA instead
    # of 8 element-gather DMAs); bqT is built from it with tiny K=1
    # matmuls, then bv overwrites the staging row.
    bqrow = constf[0:1, 8:8 + KD]
    nc.gpsimd.dma_start(out=bqrow, in_=bq[:])
    nc.vector.memset(ones_f, 1.0)
    nc.vector.memset(onesrow_f, 1.0)
    nc.vector.tensor_copy(onesb[:], ones_f)
    nc.vector.tensor_copy(onesrow_r, onesrow_f)

    # persistent attention operands (bottom of the pool stack, live to the
    # end): value/Wv stream in on the gpsimd queue from early on.
    big_pool = tc.alloc_tile_pool(name="big", bufs=1)
    vres = big_pool.tile([128, KT * VD], BF16, name="vres")     # 32KB/p
    Wvr = big_pool.tile([128, C_D * VD], BF16, name="Wvr")      # 16KB/p
    t_pool = tc.alloc_tile_pool(name="tp2", bufs=1)
    tT = t_pool.tile([128, C_D * QS], BF16, name="tT")          # 16KB/p
    k2_pool = tc.alloc_tile_pool(name="k2", bufs=1)
    keyT = k2_pool.tile([128, C_D * S], BF16, name="keyT")      # 32KB/p
    # staging rings for Wk/key rows live at the bottom of the stack: if
    # they reused just-released pool space, Tile would gate their DMAs on
    # the last PE read of that space, serializing the key stream behind
    # the whole q-chain (measured 5-8us PE stalls).
    ring_pool = tc.alloc_tile_pool(name="ring", bufs=1)

    # identity for PE transposes
    pro_pool = tc.alloc_tile_pool(name="pro", bufs=1)
    ident_f = pro_pool.tile([128, 128], F32, name="ident_f")
    ident_b = pro_pool.tile([128, 128], BF16, name="ident_b")
    masks.make_identity(nc, ident_f[:])
    nc.vector.tensor_copy(ident_b[:], ident_f[:])

    psA = tc.alloc_tile_pool(name="psA", bufs=1, space="PSUM")

    # bqT[p, c] = bq[c*128+p] via tiny K=1 matmuls from the staging row
    bq_ps = psA.tile([128, 512], F32, name="bq_ps", tag="mm", bufs=2)
    for c in range(C_KD):
        nc.tensor.matmul(bq_ps[:, c:c + 1],
                         bqrow[:, c * 128:(c + 1) * 128],
                         ones_f[0:1, 0:1],
                         start=(c == 0), stop=(c == C_KD - 1),
                         skip_group_check=True)
    nc.vector.tensor_copy(bqT, bq_ps[:, 0:8])

    # ===== P1: query transpose + q projection ============================
    qp_pool = tc.alloc_tile_pool(name="qp", bufs=1)
    qpT = qp_pool.tile([128, C_KD * QS], BF16, name="qpT")      # 16KB/p
    qt_pool = tc.alloc_tile_pool(name="qt", bufs=1)
    queryT = qt_pool.tile([128, C_D * QS], BF16, name="queryT")  # 16KB/p
    Wqr = qt_pool.tile([128, C_D * KD], BF16, name="Wqr")       # 16KB/p
    # gpsimd DMA queue order (all casting f32->bf16): Wq -> Wk -> bv ->
    # value -> Wv, matching when the PE needs each. The sync queue carries
    # only query then key, so neither stream is starved mid-prologue.
    for c in range(C_D):
        nc.gpsimd.dma_start(out=Wqr[:, c * KD:(c + 1) * KD],
                            in_=Wq[c * 128:(c + 1) * 128, :])
    wrow_all = ring_pool.tile([128, 8 * KD], BF16, name="wrow_all")
    for rt in range(D // 128):
        nc.gpsimd.dma_start(out=wrow_all[:, rt * KD:(rt + 1) * KD],
                            in_=Wk[rt * 128:(rt + 1) * 128, :])
    # bv reuses the bq staging row once bqT is built (WAR tracked by Tile)
    nc.gpsimd.dma_start(out=bv_f, in_=bv[:])
    nc.vector.tensor_copy(bv_r, bv_f)
    for kt in range(KT):
        nc.gpsimd.dma_start(out=vres[:, kt * VD:(kt + 1) * VD],
                            in_=val_b[kt * 128:(kt + 1) * 128, :])
    for c in range(C_D):
        nc.gpsimd.dma_start(out=Wvr[:, c * VD:(c + 1) * VD],
                            in_=Wv[c * 128:(c + 1) * 128, :])

    n_qrow = QS // 128
    for rt in range(n_qrow):
        qrow = qt_pool.tile([128, D], F32, name="qrow", tag="qrow", bufs=4)
        nc.sync.dma_start(out=qrow[:], in_=q_sh[rt * 128:(rt + 1) * 128, :])
        for cg in range(2):
            tp_ps = psA.tile([128, 512], F32, name="tp_ps", tag="tp", bufs=4)
            for j in range(4):
                c = cg * 4 + j
                nc.tensor.transpose(tp_ps[:, j * 128:(j + 1) * 128],
                                    qrow[:, c * 128:(c + 1) * 128], ident_f[:])
            nc.vector.tensor_copy(
                queryT[:, rt * D + cg * 512:rt * D + (cg + 1) * 512], tp_ps[:])

    qT_v = queryT[:].rearrange("p (rt x) -> p rt x", rt=n_qrow)
    for g in range(C_KD):
        for nt in range(QS // 512):
            mm_ps = psA.tile([128, 512], F32, name="mm_ps", tag="mm", bufs=2)
            for c in range(C_D):
                nc.tensor.matmul(
                    mm_ps[:],
                    Wqr[:, c * KD + g * 128:c * KD + (g + 1) * 128],
                    qT_v[:, nt * 4:(nt + 1) * 4, c * 128:(c + 1) * 128],
                    start=(c == 0), stop=(c == C_D - 1))
            nc.vector.tensor_scalar(
                out=qpT[:, g * QS + nt * 512:g * QS + (nt + 1) * 512],
                in0=mm_ps[:], scalar1=bqT[:, g:g + 1], scalar2=None, op0=add)

    # bv broadcast to all partitions via K=1 matmul
    for n in range(VD // 512):
        bc_ps = psA.tile([128, 512], F32, name="bc_ps", tag="mm", bufs=2)
        nc.tensor.matmul(bc_ps[:], onesrow_r,
                         bv_r[:, n * 512:(n + 1) * 512],
                         start=True, stop=True)
        nc.vector.tensor_copy(bvb[:, n * 512:(n + 1) * 512], bc_ps[:])
    qt_pool.release()

    # ===== P2: Wk transpose + tT = (qp Wk^T)^T ===========================
    wk_pool = tc.alloc_tile_pool(name="wk", bufs=1)
    WkT = wk_pool.tile([128, C_KD * D], BF16, name="WkT")       # 16KB/p
    n_wrow = D // 128
    for rt in range(n_wrow):
        wrow = wrow_all[:, rt * KD:(rt + 1) * KD]
        for cg in range(2):
            wtp_ps = psA.tile([128, 512], BF16, name="wtp_ps", tag="tpb",
                              bufs=2)
            for j in range(4):
                c = cg * 4 + j
                nc.tensor.transpose(wtp_ps[:, j * 128:(j + 1) * 128],
                                    wrow[:, c * 128:(c + 1) * 128], ident_b[:])
            # WkT block (kd-chunk c, d-block rt) at WkT[:, c*D + rt*128]
            dst = WkT[:].rearrange("p (c f) -> p c f", c=C_KD)[
                :, cg * 4:(cg + 1) * 4, rt * 128:(rt + 1) * 128]
            src = wtp_ps[:].rearrange("p (j f) -> p j f", j=4)
            nc.vector.tensor_copy(dst, src)

    for g2 in range(C_D):
        for nt in range(QS // 512):
            tt_ps = psA.tile([128, 512], F32, name="tt_ps", tag="mm", bufs=2)
            for c in range(C_KD):
                nc.tensor.matmul(
                    tt_ps[:],
                    WkT[:, c * D + g2 * 128:c * D + (g2 + 1) * 128],
                    qpT[:, c * QS + nt * 512:c * QS + (nt + 1) * 512],
                    start=(c == 0), stop=(c == C_KD - 1))
            nc.vector.tensor_copy(
                tT[:, g2 * QS + nt * 512:g2 * QS + (nt + 1) * 512], tt_ps[:])
    wk_pool.release()
    qp_pool.release()

    # ===== P3: key transpose =============================================
    for rt in range(KT):
        krow = ring_pool.tile([128, D], F32, name="krow", tag="krow", bufs=5)
        nc.sync.dma_start(out=krow[:],
                          in_=key_b[rt * 128:(rt + 1) * 128, :])
        for cg in range(2):
            ktp_ps = psA.tile([128, 512], F32, name="ktp_ps", tag="tp",
                              bufs=4)
            for j in range(4):
                c = cg * 4 + j
                nc.tensor.transpose(ktp_ps[:, j * 128:(j + 1) * 128],
                                    krow[:, c * 128:(c + 1) * 128], ident_f[:])
            # keyT block (d-chunk c, k-block rt) at keyT[:, c*S + rt*128]
            dst = keyT[:].rearrange("p (c f) -> p c f", c=C_D)[
                :, cg * 4:(cg + 1) * 4, rt * 128:(rt + 1) * 128]
            src = ktp_ps[:].rearrange("p (j f) -> p j f", j=4)
            nc.vector.tensor_copy(dst, src)
    pro_pool.release()
    ring_pool.release()
    psA.release()

    # ===== main attention loop ===========================================
    # PSUM: sT(2) + rs(1) + o2(4) + op(1) = 8 banks.
    psB = tc.alloc_tile_pool(name="psB", bufs=1, space="PSUM")
    sT_tiles = [psB.tile([128, QBLK], F32, name=f"sT{i}", tag=f"sT{i}")
                for i in range(2)]
    # rs_ps: cols [0:8] hold the rowsum accumulators during phase A; the
    # whole bank doubles as phase C's second accumulator (dead by then).
    rs_ps = psB.tile([128, 512], F32, name="rs_ps", tag="rs")
    o2_tiles = [psB.tile([128, QBLK], F32, name=f"o2_{i}", tag=f"o2_{i}")
                for i in range(4)]
    op_ps = psB.tile([128, 512], F32, name="op_ps", tag="opb")
    op_tiles = [op_ps, rs_ps]

    mn_pool = tc.alloc_tile_pool(name="mn", bufs=1)
    pT = mn_pool.tile([128, KT * QBLK], BF16, name="pT")        # 16KB/p
    o2T = mn_pool.tile([128, C_D * QBLK], BF16, name="o2T")     # 8KB/p
    ost_all = mn_pool.tile([128, 2 * 512], F32, name="ost_all")
    ostage_tiles = [ost_all[:, i * 512:(i + 1) * 512] for i in range(2)]

    for qb in range(NQB):
        q0 = qb * QBLK
        # ---- phase A: sT -> exp -> pT ; rowsums ----
        for kt in range(KT):
            sT_ps = sT_tiles[kt % 2]
            for g2 in range(C_D):
                nc.tensor.matmul(
                    sT_ps[:],
                    keyT[:, g2 * S + kt * 128:g2 * S + (kt + 1) * 128],
                    tT[:, g2 * QS + q0:g2 * QS + q0 + QBLK],
                    start=(g2 == 0), stop=(g2 == C_D - 1))
            nc.scalar.activation(pT[:, kt * QBLK:(kt + 1) * QBLK], sT_ps[:],
                                 Exp, scale=float(BETA))
            for qs in range(NQS):
                # single whole-bank clear on the very first rs matmul
                nc.tensor.matmul(
                    rs_ps[:, 2 * qs:2 * qs + 2],
                    pT[:, kt * QBLK + qs * 128:kt * QBLK + (qs + 1) * 128],
                    onesb[:],
                    start=(kt == 0 and qs == 0),
                    stop=(kt == KT - 1 and qs == NQS - 1),
                    skip_group_check=True)
        rrec = rrec_all[:, qb * 2 * NQS:(qb + 1) * 2 * NQS]
        nc.vector.reciprocal(rrec, rs_ps[:, 0:2 * NQS])

        # ---- phase B: o2T = value.T @ pT (direct, no transposes) ----
        for grp in range(2):
            for kt in range(KT):
                for j in range(4):
                    g2 = grp * 4 + j
                    nc.tensor.matmul(
                        o2_tiles[j][:],
                        vres[:, kt * VD + g2 * 128:kt * VD + (g2 + 1) * 128],
                        pT[:, kt * QBLK:(kt + 1) * QBLK],
                        start=(kt == 0), stop=(kt == KT - 1))
            for j in range(4):
                g2 = grp * 4 + j
                nc.vector.tensor_copy(
                    o2T[:, g2 * QBLK:(g2 + 1) * QBLK], o2_tiles[j][:])

        # ---- phase C: out = (o2T.T @ Wv) * rrec + bv ----
        for vd in range(2):
            for qs in range(NQS):
                ci = vd * NQS + qs
                acc = op_tiles[ci % 2]
                for g2 in range(C_D):
                    nc.tensor.matmul(
                        acc[:],
                        o2T[:, g2 * QBLK + qs * 128:g2 * QBLK + (qs + 1) * 128],
                        Wvr[:, g2 * VD + vd * 512:g2 * VD + (vd + 1) * 512],
                        start=(g2 == 0), stop=(g2 == C_D - 1))
                ostage = ostage_tiles[ci % 2]
                nc.vector.scalar_tensor_tensor(
                    out=ostage[:], in0=acc[:],
                    scalar=rrec[:, 2 * qs:2 * qs + 1],
                    in1=bvb[:, vd * 512:(vd + 1) * 512], op0=mult, op1=add)
                nc.sync.dma_start(
                    out=out[q0 + qs * 128:q0 + (qs + 1) * 128,
                            vd * 512:(vd + 1) * 512],
                    in_=ostage[:])

    mn_pool.release()
    psB.release()
    k2_pool.release()
    t_pool.release()
    big_pool.release()
    const_pool.release()


_NC_CACHE = {}


def _get_nc():
    if "nc" not in _NC_CACHE:
        _NC_CACHE["nc"] = build_kernel()
    return _NC_CACHE["nc"]


def kernel(query, key, value, Wq, bq, Wk, bk, Wv, bv):
    query = np.ascontiguousarray(np.asarray(query, dtype=np.float32))
    key = np.ascontiguousarray(np.asarray(key, dtype=np.float32))
    value = np.ascontiguousarray(np.asarray(value, dtype=np.float32))
    Wq = np.ascontiguousarray(np.asarray(Wq, dtype=np.float32))
    Wk = np.ascontiguousarray(np.asarray(Wk, dtype=np.float32))
    Wv = np.ascontiguousarray(np.asarray(Wv, dtype=np.float32))
    bq = np.ascontiguousarray(np.asarray(bq, dtype=np.float32))
    bk = np.ascontiguousarray(np.asarray(bk, dtype=np.float32))
    bv = np.ascontiguousarray(np.asarray(bv, dtype=np.float32))

    nc = _get_nc()
    in_maps = make_in_maps(query, key, value, Wq, bq, Wk, bk, Wv, bv)
    res = run_bass_kernel_spmd(nc, in_maps, list(range(N_CORES)))
    outp = np.empty((B, S, VD), dtype=np.float32)
    for core in range(N_CORES):
        b, h = divmod(core, 2)
        outp[b, h * QS:(h + 1) * QS, :] = res.results[core]["out"]
    return outp


def make_in_maps(query, key, value, Wq, bq, Wk, bk, Wv, bv):
    in_maps = []
    for core in range(N_CORES):
        b, h = divmod(core, 2)
        in_maps.append({
            "q_sh": np.ascontiguousarray(query[b, h * QS:(h + 1) * QS, :]),
            "key_b": key[b],
            "val_b": value[b],
            "Wq": Wq, "Wk": Wk, "Wv": Wv,
            "bq": bq, "bk": bk, "bv": bv,
        })
    return in_maps


# revision 32
# speedup vs baseline: 1.1495x; 1.1495x over previous
"""Trainium2 Bass kernel for batched dense attention.

Reference computation (per batch b):
    q = query @ Wq + bq ; k = key @ Wk + bk ; v = value @ Wv + bv
    out = softmax(BETA * q k^T) v

Shapes: query/key/value [4, 2048, 1024], weights [1024, 1024], out [4, 2048, 1024].

Sharding: 8 cores = (batch b, seq half h). Each core computes out rows
[b, h*1024:(h+1)*1024, :] from its query shard [1024, 1024] plus the full
key/value of its batch (no collectives).

Algebraic restructure vs the naive form (reaches the ideal 7.52 GMAC/core):
  - scores = (query Wq + bq) (key Wk + bk)^T. The bk term contributes a
    constant per query row, which cancels exactly in softmax, so
    scores ~ qp (Wk^T) key^T with qp = query Wq + bq. The K projection
    (which would be duplicated across the 2 cores of a batch) is replaced
    by the q-side fold tT = WkT.T @ qpT.
  - A @ value is computed against the RAW value (V projection deferred):
    out = (p.T value) Wv * (1/rowsum) + bv, with the softmax normalization
    applied at the very end (linear).

All matmuls run in bf16 (f32 PSUM accumulation): bf16 weights get the
hardware fast-weight-load path (fp32/f32r weights do not), so every
512-row matmul streams at the ~259 ns cadence floor. Measured precision
is a few 1e-3 relative, well inside the 2e-2 gate.

Dataflow (per core), all layouts chosen so matmul operands are natural:
  P1: queryT (PE transpose) ; qpT[kd,q] = Wq-chunks.T @ queryT + bq
  P2: WkT (PE transpose of Wk) ; tT[d,q] = WkT.T @ qpT
  P3: keyT[d,k] (PE transpose of key) ; value resident bf16 (natural),
      Wv resident bf16 (natural)
  A:  sT[k,q-blk] = keyT.T @ tT ; exp(BETA*sT) -> pT bf16 ;
      rowsums via tiny PE matmuls with ones
  B:  o2T[d,q-blk] = value-chunk.T @ pT (o2T produced directly, no
      transposes)
  C:  out = (o2T.T @ Wv) * (1/rowsum) + bv, DMA out
"""
import numpy as np

import concourse.bass as bass
import concourse.bacc as bacc
import concourse.tile as tile
from concourse import masks, mybir
from concourse.bass_utils import run_bass_kernel_spmd

B, S, D = 4, 2048, 1024
KD = 1024  # key_dim == value_dim == D
VD = 1024
BETA = 1.0 / float(np.sqrt(D))
N_CORES = 8
QS = S // 2  # per-core query rows (1024)

F32 = mybir.dt.float32
F32R = mybir.dt.float32r
BF16 = mybir.dt.bfloat16

C_D = D // 128     # 8 chunks over D (value cols / score contraction)
C_KD = KD // 128   # 8 chunks over KD
KT = S // 128      # 16 key tiles
QBLK = 512         # q-block size
NQB = QS // QBLK   # 2 q blocks
NQS = QBLK // 128  # 4 q slices per block


def build_kernel():
    nc = bacc.Bacc("TRN2", target_bir_lowering=False, debug=False,
                   num_devices=N_CORES)

    q_sh = nc.dram_tensor("q_sh", [QS, D], F32, kind="ExternalInput").ap()
    key_b = nc.dram_tensor("key_b", [S, D], F32, kind="ExternalInput").ap()
    val_b = nc.dram_tensor("val_b", [S, D], F32, kind="ExternalInput").ap()
    Wq = nc.dram_tensor("Wq", [D, KD], F32, kind="ExternalInput").ap()
    Wk = nc.dram_tensor("Wk", [D, KD], F32, kind="ExternalInput").ap()
    Wv = nc.dram_tensor("Wv", [D, VD], F32, kind="ExternalInput").ap()
    bq = nc.dram_tensor("bq", [KD], F32, kind="ExternalInput").ap()
    bk = nc.dram_tensor("bk", [KD], F32, kind="ExternalInput").ap()
    bv = nc.dram_tensor("bv", [VD], F32, kind="ExternalInput").ap()
    out = nc.dram_tensor("out", [QS, VD], F32, kind="ExternalOutput").ap()

    with tile.TileContext(nc) as tc:
        _body(tc, q_sh, key_b, val_b, Wq, Wk, Wv, bq, bv, out)
    nc.compile()
    return nc


def _body(tc, q_sh, key_b, val_b, Wq, Wk, Wv, bq, bv, out):
    nc = tc.nc
    Exp = mybir.ActivationFunctionType.Exp
    mult = mybir.AluOpType.mult
    add = mybir.AluOpType.add

    # ---- consolidated persistent constants ------------------------------
    # constf cols: [0:8]=bqT, [8:8+VD]=bvb, [1036:1038] ones cols,
    # row0 [1040:1168] onesrow staging, [1168:] rrec columns
    const_pool = tc.alloc_tile_pool(name="const", bufs=1)
    constf = const_pool.tile([128, 1184], F32, name="constf")
    bqT = constf[:, 0:8]
    bvb = constf[:, 8:8 + VD]
    ones_f = constf[:, 1036:1038]
    onesrow_f = constf[0:1, 1040:1040 + 128]
    bv_f = constf[0:1, 8:8 + VD]
    rrec_all = constf[:, 1168:1168 + 2 * (QS // 128)]
    # constr: row0 [0:VD]=bv_r, [1028:1028+128]=onesrow_r
    constr = const_pool.tile([128, 1184], F32R, name="constr")
    bv_r = constr[0:1, 0:VD]
    onesrow_r = constr[0:1, 1028:1028 + 128]
    onesb = const_pool.tile([128, 2], BF16, name="onesb")

    # bq stages through the bvb row-0 area first (one fast row DMA instead
    # of 8 element-gather DMAs); bqT is built from it with tiny K=1
    # matmuls, then bv overwrites the staging row.
    bqrow = constf[0:1, 8:8 + KD]
    nc.gpsimd.dma_start(out=bqrow, in_=bq[:])
    nc.vector.memset(ones_f, 1.0)
    nc.vector.memset(onesrow_f, 1.0)
    nc.vector.tensor_copy(onesb[:], ones_f)
    nc.vector.tensor_copy(onesrow_r, onesrow_f)

    # persistent attention operands (bottom of the pool stack, live to the
    # end): value/Wv stream in on the gpsimd queue from early on.
    big_pool = tc.alloc_tile_pool(name="big", bufs=1)
    vres = big_pool.tile([128, KT * VD], BF16, name="vres")     # 32KB/p
    Wvr = big_pool.tile([128, C_D * VD], BF16, name="Wvr")      # 16KB/p
    t_pool = tc.alloc_tile_pool(name="tp2", bufs=1)
    tT = t_pool.tile([128, C_D * QS], BF16, name="tT")          # 16KB/p
    k2_pool = tc.alloc_tile_pool(name="k2", bufs=1)
    keyT = k2_pool.tile([128, C_D * S], BF16, name="keyT")      # 32KB/p
    # staging rings for Wk/key rows live at the bottom of the stack: if
    # they reused just-released pool space, Tile would gate their DMAs on
    # the last PE read of that space, serializing the key stream behind
    # the whole q-chain (measured 5-8us PE stalls).
    ring_pool = tc.alloc_tile_pool(name="ring", bufs=1)

    # identity for PE transposes
    pro_pool = tc.alloc_tile_pool(name="pro", bufs=1)
    ident_f = pro_pool.tile([128, 128], F32, name="ident_f")
    ident_b = pro_pool.tile([128, 128], BF16, name="ident_b")
    masks.make_identity(nc, ident_f[:])
    nc.vector.tensor_copy(ident_b[:], ident_f[:])

    psA = tc.alloc_tile_pool(name="psA", bufs=1, space="PSUM")

    # ===== P1: query transpose + q projection ============================
    qp_pool = tc.alloc_tile_pool(name="qp", bufs=1)
    qpT = qp_pool.tile([128, C_KD * QS], BF16, name="qpT")      # 16KB/p
    qt_pool = tc.alloc_tile_pool(name="qt", bufs=1)
    queryT = qt_pool.tile([128, C_D * QS], BF16, name="queryT")  # 16KB/p
    Wqr = qt_pool.tile([128, C_D * KD], BF16, name="Wqr")       # 16KB/p
    # gpsimd DMA queue order (all casting f32->bf16): Wq -> Wk -> bv ->
    # value -> Wv, matching when the PE needs each. The sync queue carries
    # only query then key, so neither stream is starved mid-prologue.
    for c in range(C_D):
        nc.gpsimd.dma_start(out=Wqr[:, c * KD:(c + 1) * KD],
                            in_=Wq[c * 128:(c + 1) * 128, :])
    wrow_all = ring_pool.tile([128, 8 * KD], BF16, name="wrow_all")
    for rt in range(D // 128):
        nc.gpsimd.dma_start(out=wrow_all[:, rt * KD:(rt + 1) * KD],
                            in_=Wk[rt * 128:(rt + 1) * 128, :])
    # bv reuses the bq staging row once bqT is built (WAR tracked by Tile)
    nc.gpsimd.dma_start(out=bv_f, in_=bv[:])
    nc.vector.tensor_copy(bv_r, bv_f)
    # key rows (bf16 cast): an 8-deep ring, so half the key streams in
    # un-gated before the kt-th transpose consumption paces the rest
    krows = [ring_pool.tile([128, D], BF16, name="krow", tag="krow", bufs=8)
             for _ in range(KT)]
    for rt in range(KT):
        nc.gpsimd.dma_start(out=krows[rt][:],
                            in_=key_b[rt * 128:(rt + 1) * 128, :])
    for kt in range(KT):
        nc.gpsimd.dma_start(out=vres[:, kt * VD:(kt + 1) * VD],
                            in_=val_b[kt * 128:(kt + 1) * 128, :])
    for c in range(C_D):
        nc.gpsimd.dma_start(out=Wvr[:, c * VD:(c + 1) * VD],
                            in_=Wv[c * 128:(c + 1) * 128, :])

    n_qrow = QS // 128
    for rt in range(n_qrow):
        qrow = qt_pool.tile([128, D], F32, name="qrow", tag="qrow", bufs=4)
        nc.sync.dma_start(out=qrow[:], in_=q_sh[rt * 128:(rt + 1) * 128, :])
        for cg in range(2):
            tp_ps = psA.tile([128, 512], F32, name="tp_ps", tag="tp", bufs=4)
            for j in range(4):
                c = cg * 4 + j
                nc.tensor.transpose(tp_ps[:, j * 128:(j + 1) * 128],
                                    qrow[:, c * 128:(c + 1) * 128], ident_f[:])
            nc.vector.tensor_copy(
                queryT[:, rt * D + cg * 512:rt * D + (cg + 1) * 512], tp_ps[:])

    # bqT[p, c] = bq[c*128+p] via tiny K=1 matmuls from the staging row
    # (emitted after the transposes so the PE starts on real work ASAP)
    bq_ps = psA.tile([128, 512], F32, name="bq_ps", tag="mm", bufs=2)
    for c in range(C_KD):
        nc.tensor.matmul(bq_ps[:, c:c + 1],
                         bqrow[:, c * 128:(c + 1) * 128],
                         ones_f[0:1, 0:1],
                         start=(c == 0), stop=(c == C_KD - 1),
                         skip_group_check=True)
    nc.vector.tensor_copy(bqT, bq_ps[:, 0:8])

    qT_v = queryT[:].rearrange("p (rt x) -> p rt x", rt=n_qrow)
    for g in range(C_KD):
        for nt in range(QS // 512):
            mm_ps = psA.tile([128, 512], F32, name="mm_ps", tag="mm", bufs=2)
            for c in range(C_D):
                nc.tensor.matmul(
                    mm_ps[:],
                    Wqr[:, c * KD + g * 128:c * KD + (g + 1) * 128],
                    qT_v[:, nt * 4:(nt + 1) * 4, c * 128:(c + 1) * 128],
                    start=(c == 0), stop=(c == C_D - 1))
            nc.vector.tensor_scalar(
                out=qpT[:, g * QS + nt * 512:g * QS + (nt + 1) * 512],
                in0=mm_ps[:], scalar1=bqT[:, g:g + 1], scalar2=None, op0=add)

    # bv broadcast to all partitions via K=1 matmul
    for n in range(VD // 512):
        bc_ps = psA.tile([128, 512], F32, name="bc_ps", tag="mm", bufs=2)
        nc.tensor.matmul(bc_ps[:], onesrow_r,
                         bv_r[:, n * 512:(n + 1) * 512],
                         start=True, stop=True)
        nc.vector.tensor_copy(bvb[:, n * 512:(n + 1) * 512], bc_ps[:])
    qt_pool.release()

    # ===== P2: Wk transpose + tT = (qp Wk^T)^T ===========================
    wk_pool = tc.alloc_tile_pool(name="wk", bufs=1)
    WkT = wk_pool.tile([128, C_KD * D], BF16, name="WkT")       # 16KB/p
    n_wrow = D // 128
    for rt in range(n_wrow):
        wrow = wrow_all[:, rt * KD:(rt + 1) * KD]
        for cg in range(2):
            wtp_ps = psA.tile([128, 512], BF16, name="wtp_ps", tag="tpb",
                              bufs=2)
            for j in range(4):
                c = cg * 4 + j
                nc.tensor.transpose(wtp_ps[:, j * 128:(j + 1) * 128],
                                    wrow[:, c * 128:(c + 1) * 128], ident_b[:])
            # WkT block (kd-chunk c, d-block rt) at WkT[:, c*D + rt*128]
            dst = WkT[:].rearrange("p (c f) -> p c f", c=C_KD)[
                :, cg * 4:(cg + 1) * 4, rt * 128:(rt + 1) * 128]
            src = wtp_ps[:].rearrange("p (j f) -> p j f", j=4)
            nc.vector.tensor_copy(dst, src)

    for g2 in range(C_D):
        for nt in range(QS // 512):
            tt_ps = psA.tile([128, 512], F32, name="tt_ps", tag="mm", bufs=2)
            for c in range(C_KD):
                nc.tensor.matmul(
                    tt_ps[:],
                    WkT[:, c * D + g2 * 128:c * D + (g2 + 1) * 128],
                    qpT[:, c * QS + nt * 512:c * QS + (nt + 1) * 512],
                    start=(c == 0), stop=(c == C_KD - 1))
            nc.vector.tensor_copy(
                tT[:, g2 * QS + nt * 512:g2 * QS + (nt + 1) * 512], tt_ps[:])
    wk_pool.release()
    qp_pool.release()

    # ===== P3: key transpose (bf16) ======================================
    for rt in range(KT):
        krow = krows[rt]
        for cg in range(2):
            ktp_ps = psA.tile([128, 512], BF16, name="ktp_ps", tag="tpb",
                              bufs=2)
            for j in range(4):
                c = cg * 4 + j
                nc.tensor.transpose(ktp_ps[:, j * 128:(j + 1) * 128],
                                    krow[:, c * 128:(c + 1) * 128], ident_b[:])
            # keyT block (d-chunk c, k-block rt) at keyT[:, c*S + rt*128]
            dst = keyT[:].rearrange("p (c f) -> p c f", c=C_D)[
                :, cg * 4:(cg + 1) * 4, rt * 128:(rt + 1) * 128]
            src = ktp_ps[:].rearrange("p (j f) -> p j f", j=4)
            nc.vector.tensor_copy(dst, src)
    pro_pool.release()
    ring_pool.release()
    psA.release()

    # ===== main attention loop ===========================================
    # PSUM: sT(2) + rs(1) + o2(4) + op(1) = 8 banks.
    psB = tc.alloc_tile_pool(name="psB", bufs=1, space="PSUM")
    sT_tiles = [psB.tile([128, QBLK], F32, name=f"sT{i}", tag=f"sT{i}")
                for i in range(2)]
    # rs_ps: cols [0:8] hold the rowsum accumulators during phase A; the
    # whole bank doubles as phase C's second accumulator (dead by then).
    rs_ps = psB.tile([128, 512], F32, name="rs_ps", tag="rs")
    o2_tiles = [psB.tile([128, QBLK], F32, name=f"o2_{i}", tag=f"o2_{i}")
                for i in range(4)]
    op_ps = psB.tile([128, 512], F32, name="op_ps", tag="opb")
    op_tiles = [op_ps, rs_ps]

    mn_pool = tc.alloc_tile_pool(name="mn", bufs=1)
    pT = mn_pool.tile([128, KT * QBLK], BF16, name="pT")        # 16KB/p
    o2T = mn_pool.tile([128, C_D * QBLK], BF16, name="o2T")     # 8KB/p
    ost_all = mn_pool.tile([128, 2 * 512], F32, name="ost_all")
    ostage_tiles = [ost_all[:, i * 512:(i + 1) * 512] for i in range(2)]

    for qb in range(NQB):
        q0 = qb * QBLK
        # ---- phase A: sT -> exp -> pT ; rowsums ----
        def emit_rowsums(kt):
            # rowsums for tile kt run one iteration late so the PE never
            # waits on exp(kt); single whole-bank clear on the first one
            for qs in range(NQS):
                nc.tensor.matmul(
                    rs_ps[:, 2 * qs:2 * qs + 2],
                    pT[:, kt * QBLK + qs * 128:kt * QBLK + (qs + 1) * 128],
                    onesb[:],
                    start=(kt == 0 and qs == 0),
                    stop=(kt == KT - 1 and qs == NQS - 1),
                    skip_group_check=True)

        for kt in range(KT):
            sT_ps = sT_tiles[kt % 2]
            for g2 in range(C_D):
                nc.tensor.matmul(
                    sT_ps[:],
                    keyT[:, g2 * S + kt * 128:g2 * S + (kt + 1) * 128],
                    tT[:, g2 * QS + q0:g2 * QS + q0 + QBLK],
                    start=(g2 == 0), stop=(g2 == C_D - 1))
            nc.scalar.activation(pT[:, kt * QBLK:(kt + 1) * QBLK], sT_ps[:],
                                 Exp, scale=float(BETA))
            if kt > 0:
                emit_rowsums(kt - 1)
        emit_rowsums(KT - 1)
        rrec = rrec_all[:, qb * 2 * NQS:(qb + 1) * 2 * NQS]
        nc.vector.reciprocal(rrec, rs_ps[:, 0:2 * NQS])

        # ---- phase B: o2T = value.T @ pT (direct, no transposes) ----
        for grp in range(2):
            for kt in range(KT):
                for j in range(4):
                    g2 = grp * 4 + j
                    nc.tensor.matmul(
                        o2_tiles[j][:],
                        vres[:, kt * VD + g2 * 128:kt * VD + (g2 + 1) * 128],
                        pT[:, kt * QBLK:(kt + 1) * QBLK],
                        start=(kt == 0), stop=(kt == KT - 1))
            for j in range(4):
                g2 = grp * 4 + j
                nc.vector.tensor_copy(
                    o2T[:, g2 * QBLK:(g2 + 1) * QBLK], o2_tiles[j][:])

        # ---- phase C: out = (o2T.T @ Wv) * rrec + bv ----
        for vd in range(2):
            for qs in range(NQS):
                ci = vd * NQS + qs
                acc = op_tiles[ci % 2]
                for g2 in range(C_D):
                    nc.tensor.matmul(
                        acc[:],
                        o2T[:, g2 * QBLK + qs * 128:g2 * QBLK + (qs + 1) * 128],
                        Wvr[:, g2 * VD + vd * 512:g2 * VD + (vd + 1) * 512],
                        start=(g2 == 0), stop=(g2 == C_D - 1))
                ostage = ostage_tiles[ci % 2]
                nc.vector.scalar_tensor_tensor(
                    out=ostage[:], in0=acc[:],
                    scalar=rrec[:, 2 * qs:2 * qs + 1],
                    in1=bvb[:, vd * 512:(vd + 1) * 512], op0=mult, op1=add)
                nc.sync.dma_start(
                    out=out[q0 + qs * 128:q0 + (qs + 1) * 128,
                            vd * 512:(vd + 1) * 512],
                    in_=ostage[:])

    mn_pool.release()
    psB.release()
    k2_pool.release()
    t_pool.release()
    big_pool.release()
    const_pool.release()


_NC_CACHE = {}


def _get_nc():
    if "nc" not in _NC_CACHE:
        _NC_CACHE["nc"] = build_kernel()
    return _NC_CACHE["nc"]


def kernel(query, key, value, Wq, bq, Wk, bk, Wv, bv):
    query = np.ascontiguousarray(np.asarray(query, dtype=np.float32))
    key = np.ascontiguousarray(np.asarray(key, dtype=np.float32))
    value = np.ascontiguousarray(np.asarray(value, dtype=np.float32))
    Wq = np.ascontiguousarray(np.asarray(Wq, dtype=np.float32))
    Wk = np.ascontiguousarray(np.asarray(Wk, dtype=np.float32))
    Wv = np.ascontiguousarray(np.asarray(Wv, dtype=np.float32))
    bq = np.ascontiguousarray(np.asarray(bq, dtype=np.float32))
    bk = np.ascontiguousarray(np.asarray(bk, dtype=np.float32))
    bv = np.ascontiguousarray(np.asarray(bv, dtype=np.float32))

    nc = _get_nc()
    in_maps = make_in_maps(query, key, value, Wq, bq, Wk, bk, Wv, bv)
    res = run_bass_kernel_spmd(nc, in_maps, list(range(N_CORES)))
    outp = np.empty((B, S, VD), dtype=np.float32)
    for core in range(N_CORES):
        b, h = divmod(core, 2)
        outp[b, h * QS:(h + 1) * QS, :] = res.results[core]["out"]
    return outp


def make_in_maps(query, key, value, Wq, bq, Wk, bk, Wv, bv):
    in_maps = []
    for core in range(N_CORES):
        b, h = divmod(core, 2)
        in_maps.append({
            "q_sh": np.ascontiguousarray(query[b, h * QS:(h + 1) * QS, :]),
            "key_b": key[b],
            "val_b": value[b],
            "Wq": Wq, "Wk": Wk, "Wv": Wv,
            "bq": bq, "bk": bk, "bv": bv,
        })
    return in_maps
